# revision 13
# baseline (speedup 1.0000x reference)
"""Trainium2 Bass kernel for nn_Encoder (2-layer LSTM encoder).

Strategy: tensor-parallel over the 1024 LSTM units across 8 cores.
Each core owns 128 units (512 of the 4096 gate columns, gate-major
[i|f|g|o]). Per recurrence step each core computes its gate slice
z^T = U_s^T @ h^T  as 8 K-tiles x 4 M-tiles of [128x128]x[128,64]
matmuls, applies the gate nonlinearities in [units, batch] layout,
and broadcasts its new h-shard [128,64] to all peers via remote_dma
(XOR slot addressing, so per-core weights are row-permuted to match).
The input projections (x@W per layer) are done as big parallel GEMMs
before each recurrence, sharded by gate columns (each core computes
its own 512 gate cols for all 16384 (t,b) positions).
"""
import os
import sys
import types
import numpy as np
import ml_dtypes

sys.path.insert(0, '/opt/trn_rl_repo')

from concourse import bass, bass_utils, tile, mybir, bacc  # noqa: E402
from concourse.tile import add_dep_helper  # noqa: E402
import bass_rust  # noqa: E402
from concourse.vector_clock import ScopedClock  # noqa: E402

B, T, E, U, V = 64, 256, 512, 1024, 32000
NCORES = 8
G = 4  # gates
UPC = U // NCORES          # units per core = 128
MPC = G * UPC // 128       # M tiles per core = 4
BT = B * T                 # 16384
AF = mybir.ActivationFunctionType
bf16 = mybir.dt.bfloat16
f32 = mybir.dt.float32


# ---------------------------------------------------------------- tile patch
def _patched_drain_and_barrier(self, tick_clock, wait_clock):
    nc = self.nc
    nops = [nc.sync.nop(nofuse=True, hint=f"drain_wait_{i}") for i in range(24)]
    drain_inst = nc.sync.drain()
    wait_clock.add_sem_waits(
        drain_inst.ins, ScopedClock({None: tick_clock.global_clock})
    )
    si = drain_inst.ins.sync_info
    if si is not None and si.on_wait and len(si.on_wait) > 1:
        waits = list(si.on_wait)
        extra, keep = waits[:-1], waits[-1:]
        assert len(extra) <= len(nops), f"too many drain waits: {len(waits)}"
        for nop, w in zip(nops, extra):
            nop.ins.sync_info = bass_rust.SyncInfo(on_wait=[w], on_update=[])
        drain_inst.ins.sync_info = bass_rust.SyncInfo(
            on_wait=keep, on_update=list(si.on_update)
        )
    nc.all_engine_barrier()
    assert self.sems is not None
    popped = nc._tile_sem_poison_stack.pop()
    assert popped is self._sem_poison
    nc.clear_and_free_semaphores(list(self.sems.allocated().values()))
    nc.all_engine_barrier()


tile.TileContext._drain_and_barrier = _patched_drain_and_barrier


def _install_ntff_hook():
    """Recreate the missing antenv.axon_hooks so trace=True profiling works."""
    try:
        import antenv
        if 'antenv.axon_hooks' in sys.modules:
            return
        mod = types.ModuleType('antenv.axon_hooks')
        holder = [None]
        mod.set_axon_ntff_profile_hook = lambda h: holder.__setitem__(0, h)
        mod.get_axon_ntff_profile_hook = lambda: holder[0]
        sys.modules['antenv.axon_hooks'] = mod
        antenv.axon_hooks = mod
        from trn_agent_boot.trn_boot import _ntff_profile_via_ctypes
        mod.set_axon_ntff_profile_hook(
            _ntff_profile_via_ctypes('/opt/axon/libaxon_pjrt.so'))
    except Exception:
        pass


# ---------------------------------------------------------------- builder
class Enc:
    """Builds the SPMD device program."""

    def __init__(self, t_steps=T, hw=True, debug=False, dbg_out=False):
        self.T = t_steps
        self.hw = hw  # apply cross-die xor2 routing compensation
        self.dbg_out = dbg_out
        self.deferred = []  # (inst, sem, value) waits injected post-schedule
        nc = self.nc = bacc.Bacc("TRN2", target_bir_lowering=False,
                                 debug=debug, num_devices=NCORES)
        BT_ = B * self.T
        dt = nc.dram_tensor
        # inputs (per-core values differ; same names/shapes)
        self.emb = dt("emb", [V, E], bf16, kind="ExternalInput")
        self.toki = dt("toki", [128, BT_ // 16], mybir.dt.int16, kind="ExternalInput")
        self.w0 = dt("w0", [E // 128, 128, G * UPC], bf16, kind="ExternalInput")
        self.w1 = dt("w1", [U // 128, 128, G * UPC], bf16, kind="ExternalInput")
        self.u0 = dt("u0", [U // 128, 128, G * UPC], bf16, kind="ExternalInput")
        self.u1 = dt("u1", [U // 128, 128, G * UPC], bf16, kind="ExternalInput")
        self.b0 = dt("b0", [128, MPC], f32, kind="ExternalInput")
        self.b1 = dt("b1", [128, MPC], f32, kind="ExternalInput")
        self.ident = dt("ident", [128, 128], bf16, kind="ExternalInput")
        # internal DRAM (ExternalOutput in debug builds for inspection)
        ik = dict(kind="ExternalOutput") if dbg_out else {}
        self.xz0 = dt("xz0", [MPC, 128, BT_], f32, **ik)
        self.x1t = dt("x1t", [NCORES, 128, BT_], bf16, **ik)
        self.xz1 = dt("xz1", [MPC, 128, BT_], f32, **ik)
        # outputs
        self.oseq = dt("oseq", [B, self.T, UPC], f32, kind="ExternalOutput")
        self.oc = dt("oc", [128, B], f32, kind="ExternalOutput")

        # static SBUF
        sb = nc.alloc_sbuf_tensor
        self.u_sb = [sb("u0_sb", [128, NCORES, G * UPC], bf16),
                     sb("u1_sb", [128, NCORES, G * UPC], bf16)]
        self.w0_sb = sb("w0_sb", [128, E // 128, G * UPC], bf16)
        self.w1_sb = sb("w1_sb", [128, U // 128, G * UPC], bf16)
        self.b_sb = [sb("b0_sb", [128, MPC], f32), sb("b1_sb", [128, MPC], f32)]
        self.id_sb = sb("id_sb", [128, 128], bf16)
        self.toki_sb = sb("toki_sb", [128, (B * self.T) // 16], mybir.dt.int16)
        self.hs = sb("hs", [128, 2, NCORES, B], bf16)   # [u, parity, slot, b]
        self.cs = sb("cs", [128, 2, B], f32)
        # cross-core sems
        self.rsem = [[nc.alloc_semaphore(f"rs{l}_{d}") for d in range(8)]
                     for l in range(2)]
        self.lsem = [nc.alloc_semaphore("ls0"), nc.alloc_semaphore("ls1")]

    def wait_carrier(self, engine, sem, val, anchor, hint):
        n = engine.nop(nofuse=True, hint=hint)
        if anchor is not None:
            add_dep_helper(n.ins, anchor.ins, sync=False, reason="carrier-anchor")
        self.deferred.append((n, sem, val))
        return n

    # ------------------------------------------------------------ phases
    def load_consts(self):
        nc = self.nc
        nc.sync.dma_start(out=self.u_sb[0].ap(),
                          in_=self.u0.ap().rearrange("k p m -> p k m"))
        nc.sync.dma_start(out=self.u_sb[1].ap(),
                          in_=self.u1.ap().rearrange("k p m -> p k m"))
        nc.sync.dma_start(out=self.w0_sb.ap(),
                          in_=self.w0.ap().rearrange("k p m -> p k m"))
        nc.sync.dma_start(out=self.w1_sb.ap(),
                          in_=self.w1.ap().rearrange("k p m -> p k m"))
        nc.sync.dma_start(out=self.b_sb[0].ap(), in_=self.b0.ap())
        nc.sync.dma_start(out=self.b_sb[1].ap(), in_=self.b1.ap())
        nc.sync.dma_start(out=self.id_sb.ap(), in_=self.ident.ap())
        nc.sync.dma_start(out=self.toki_sb.ap(), in_=self.toki.ap())
        nc.vector.memset(self.hs.ap()[:, 0, :, :], 0.0)
        nc.vector.memset(self.cs.ap()[:, 0, :], 0.0)

    def phase_in_proj(self, ctx, tc, layer):
        """xz[l]^T[m][128, BT] = W^T x^T + b for this core's gate cols."""
        nc = self.nc
        BT_ = B * self.T
        CH = min(512, BT_)
        nch = BT_ // CH
        kt = E // 128 if layer == 0 else U // 128
        w_sb = self.w0_sb if layer == 0 else self.w1_sb
        xz = self.xz0 if layer == 0 else self.xz1
        rhs_pool = ctx.enter_context(tc.tile_pool(name=f"ph{layer}_rhs", bufs=3))
        out_pool = ctx.enter_context(tc.tile_pool(name=f"ph{layer}_out", bufs=2))
        ps_pool = ctx.enter_context(
            tc.tile_pool(name=f"ph{layer}_ps", bufs=2, space="PSUM"))
        for c in range(nch):
            if layer == 0:
                rhs = rhs_pool.tile([128, kt, CH], bf16, tag="rhs")
                nc.gpsimd.dma_gather(
                    out_ap=rhs[:, :, :],
                    in_ap=self.emb.ap(),
                    idxs_ap=self.toki_sb.ap()[:, (CH // 16) * c:(CH // 16) * (c + 1)],
                    num_idxs=CH,
                    num_idxs_reg=CH,
                    elem_size=E,
                    transpose=True,
                )
            else:
                rhs = rhs_pool.tile([128, kt, CH], bf16, tag="rhs")
                nc.sync.dma_start(
                    out=rhs[:, :, :],
                    in_=self.x1t.ap().rearrange("k p n -> p k n")[:, :, CH * c:CH * (c + 1)],
                )
            out_t = out_pool.tile([128, MPC, CH], f32, tag="out")
            for m in range(MPC):
                ps = ps_pool.tile([128, CH], f32, tag=f"ps{m}", name=f"ps{m}")
                for k in range(kt):
                    nc.tensor.matmul(
                        ps[:, :], w_sb.ap()[:, k, 128 * m:128 * (m + 1)],
                        rhs[:, k, :], start=(k == 0), stop=(k == kt - 1))
                nc.scalar.activation(out_t[:, m, :], ps[:, :], AF.Identity,
                                     bias=self.b_sb[layer].ap()[:, m:m + 1])
            nc.sync.dma_start(
                out=xz.ap()[:, :, CH * c:CH * (c + 1)].rearrange("m p n -> p m n"),
                in_=out_t[:, :, :])

    def recurrence(self, ctx, tc, layer):
        nc = self.nc
        Tn = self.T
        xz = self.xz0 if layer == 0 else self.xz1
        u_sb = self.u_sb[layer]
        rsem, lsem = self.rsem[layer], self.lsem[layer]
        hs, cs = self.hs.ap(), self.cs.ap()

        xz_pool = ctx.enter_context(tc.tile_pool(name=f"r{layer}_xz", bufs=4))
        z_pool = ctx.enter_context(tc.tile_pool(name=f"r{layer}_z", bufs=2))
        a_pool = ctx.enter_context(tc.tile_pool(name=f"r{layer}_a", bufs=2))
        t_pool = ctx.enter_context(tc.tile_pool(name=f"r{layer}_t", bufs=2))
        o_pool = ctx.enter_context(tc.tile_pool(name=f"r{layer}_o", bufs=3))
        ps_pool = ctx.enter_context(
            tc.tile_pool(name=f"r{layer}_ps", bufs=2, space="PSUM"))
        tr_pool = ctx.enter_context(
            tc.tile_pool(name=f"r{layer}_tr", bufs=2, space="PSUM"))

        last_pe = None       # anchor chain on PE
        last_dve_w = None    # anchor for DVE lsem carrier
        last_sp = None       # last sync-engine instruction (anchor)
        for t in range(Tn):
            par, nxt = t % 2, (t + 1) % 2
            xz_t = xz_pool.tile([128, MPC, B], f32, tag="xz")
            nc.sync.dma_start(out=xz_t[:, :, :],
                              in_=xz.ap()[:, :, B * t:B * (t + 1)].rearrange("m p n -> p m n"))
            ps = ps_pool.tile([128, MPC * B], f32, tag="ps")
            pend_carrier = False
            for m in range(MPC):
                for d in range(NCORES):
                    if m == 0 and d > 0 and t > 0:
                        w = self.wait_carrier(nc.tensor, rsem[d], 2 * t, last_pe,
                                              f"r{layer}t{t}d{d}")
                        last_pe = w
                        pend_carrier = True
                    mm = nc.tensor.matmul(
                        ps[:, B * m:B * (m + 1)],
                        u_sb.ap()[:, d, 128 * m:128 * (m + 1)],
                        hs[:, par, d, :],
                        start=(d == 0), stop=(d == NCORES - 1))
                    if pend_carrier:
                        add_dep_helper(mm.ins, last_pe.ins, sync=False,
                                       reason="mm-after-carrier")
                        pend_carrier = False
                    last_pe = mm
            # z = psum + xz ; i,f,g,o each [128, B]
            z = z_pool.tile([128, MPC * B], f32, tag="z")
            nc.vector.tensor_add(z[:, :], ps[:, :],
                                 xz_t[:, :, :].rearrange("p m n -> p (m n)"))
            a = a_pool.tile([128, MPC * B], f32, tag="a")
            nc.scalar.activation(a[:, 0:2 * B], z[:, 0:2 * B], AF.Sigmoid)
            nc.scalar.activation(a[:, 2 * B:3 * B], z[:, 2 * B:3 * B], AF.Tanh)
            nc.scalar.activation(a[:, 3 * B:4 * B], z[:, 3 * B:4 * B], AF.Sigmoid)
            t1 = t_pool.tile([128, B], f32, tag="t1")
            t2 = t_pool.tile([128, B], f32, tag="t2")
            tc_t = t_pool.tile([128, B], f32, tag="tc")
            nc.vector.tensor_mul(t1[:, :], a[:, B:2 * B], cs[:, par, :])
            nc.vector.tensor_mul(t2[:, :], a[:, 0:B], a[:, 2 * B:3 * B])
            nc.vector.tensor_add(cs[:, nxt, :], t1[:, :], t2[:, :])
            nc.scalar.activation(tc_t[:, :], cs[:, nxt, :], AF.Tanh)
            if t >= 2:
                # protect send source buffer from overwrite while in flight
                w = self.wait_carrier(nc.vector, lsem, 112 * (t - 1),
                                      last_dve_w, f"l{layer}ls{t}")
                hw_inst = nc.vector.tensor_mul(hs[:, nxt, 0, :], a[:, 3 * B:4 * B],
                                               tc_t[:, :])
                add_dep_helper(hw_inst.ins, w.ins, sync=False, reason="h-after-lsem")
            else:
                hw_inst = nc.vector.tensor_mul(hs[:, nxt, 0, :], a[:, 3 * B:4 * B],
                                               tc_t[:, :])
            last_dve_w = hw_inst
            # layer0: store gathered h^{t} (parity par, all slots valid after
            # this step's MMs) to x1t. The trigger below is gated on this
            # store so no peer's h^{t+2} can land before it drains.
            st = None
            if layer == 0 and t >= 1:
                st = nc.sync.dma_start(
                    out=self.x1t.ap()[:, :, B * (t - 1):B * t].rearrange("k p n -> p k n"),
                    in_=hs[:, par, :, :])
                add_dep_helper(st.ins, last_pe.ins, sync=True,
                               reason="x1t-after-mms")
                last_sp = st
            # broadcast own shard to peers' slot d
            for d in range(1, NCORES):
                rd = [None] * 8
                tgt = (d ^ 2) if (self.hw and d >= 4) else d
                rd[d] = (0, tgt)
                nc.gpsimd.remote_dma_broadcast(
                    out_ap=hs[:, nxt, d, :], in_ap=hs[:, nxt, 0, :],
                    remote_sem=rsem[d], local_sem=lsem, rdests=rd)
            trig = nc.gpsimd.trigger_dma(count=None)
            if st is not None:
                add_dep_helper(trig.ins, st.ins, sync=True,
                               reason="sends-after-x1t-store")
            if layer != 0:
                # transpose own new h-shard -> [B, UPC] and store to oseq
                tr = tr_pool.tile([B, 128], bf16, tag="tr")
                nc.tensor.transpose(tr[:, :], hs[:, nxt, 0, :], self.id_sb.ap())
                ot = o_pool.tile([B, 128], f32, tag="ot")
                nc.vector.tensor_copy(ot[:, :], tr[:, :])
                nc.sync.dma_start(out=self.oseq.ap()[:, t, :], in_=ot[:, :])

        # final gathered h^{T} for x1t (layer 0) needs the last exchange
        if layer == 0:
            par = Tn % 2
            anchor = last_sp
            for d in range(1, NCORES):
                anchor = self.wait_carrier(nc.sync, self.rsem[0][d], 2 * Tn,
                                           anchor, f"fin0_{d}")
            st = nc.sync.dma_start(
                out=self.x1t.ap()[:, :, B * (Tn - 1):B * Tn].rearrange("k p n -> p k n"),
                in_=hs[:, par, :, :])
            add_dep_helper(st.ins, anchor.ins, sync=False, reason="fin-x1t")
            # reset state for layer 1 (after the store)
            ms1 = nc.vector.memset(hs[:, 0, :, :], 0.0)
            add_dep_helper(ms1.ins, st.ins, sync=True, reason="reset-after-store")
            nc.vector.memset(cs[:, 0, :], 0.0)
        else:
            par = Tn % 2
            nc.sync.dma_start(out=self.oc.ap(), in_=cs[:, par, :])

    def build(self):
        from contextlib import ExitStack
        nc = self.nc
        with tile.TileContext(nc) as tc:
            self.load_consts()
            with ExitStack() as ctx:
                self.phase_in_proj(ctx, tc, 0)
            with ExitStack() as ctx:
                self.recurrence(ctx, tc, 0)
            with ExitStack() as ctx:
                self.phase_in_proj(ctx, tc, 1)
            with ExitStack() as ctx:
                self.recurrence(ctx, tc, 1)
        for inst, sem, val in self.deferred:
            if val > 0:
                bass_rust.wait_op(inst.ins, sem, val, "sem-ge", True)
        nc.compile()
        return nc


# ---------------------------------------------------------------- host side
def _prep_inputs(tokens, emb, W0, U0, b0, W1, U1, b1, t_steps=T):
    tokens = np.asarray(tokens)
    emb = np.asarray(emb, np.float32)
    W0 = np.asarray(W0, np.float32); U0 = np.asarray(U0, np.float32)
    W1 = np.asarray(W1, np.float32); U1 = np.asarray(U1, np.float32)
    b0 = np.asarray(b0, np.float32); b1 = np.asarray(b1, np.float32)
    emb_bf = emb.astype(ml_dtypes.bfloat16)
    seq = tokens[:, :t_steps].T.reshape(-1).astype(np.int16)  # col j=(t*B+b)
    toki = np.zeros((128, seq.size // 16), np.int16)
    toki[:16] = seq.reshape(-1, 16).T
    toki[16:32] = toki[:16]  # HW SWDGE ucode reads rows 16-31; sim reads 0-15
    ident = np.eye(128, dtype=ml_dtypes.bfloat16)

    def gate_cols(s):
        # column indices of core s's 512 gate cols, gate-major
        return np.concatenate(
            [np.arange(g * U + UPC * s, g * U + UPC * (s + 1)) for g in range(G)])

    in_maps = []
    for s in range(NCORES):
        cols = gate_cols(s)
        w0s = W0[:, cols]                      # [512, 512]
        w0t = np.ascontiguousarray(
            w0s.reshape(E // 128, 128, G * UPC)).astype(ml_dtypes.bfloat16)
        # xor-permuted K tiles for U and W1
        def xperm(Wm):
            out = np.empty((U // 128, 128, G * UPC), np.float32)
            for j in range(NCORES):
                src = Wm[UPC * (s ^ j):UPC * ((s ^ j) + 1), cols]
                out[j] = src
            return out.astype(ml_dtypes.bfloat16)
        u0s = xperm(U0)
        u1s = xperm(U1)
        w1s = xperm(W1)
        b0s = np.ascontiguousarray(b0[cols].reshape(G, UPC).T.reshape(128, MPC),)
        # careful: cols grouped gate-major: b[cols] = [i(128)|f|g|o];
        # b_sb[u, m] = bias of gate m unit u  -> reshape(G, UPC).T is [UPC, G]
        b1s = np.ascontiguousarray(b1[cols].reshape(G, UPC).T.reshape(128, MPC))
        in_maps.append({
            "emb": emb_bf, "toki": toki, "ident": ident,
            "w0": w0t, "w1": w1s, "u0": u0s, "u1": u1s,
            "b0": b0s.astype(np.float32), "b1": b1s.astype(np.float32),
        })
    return in_maps


_CACHE = {}


def _get_module(t_steps=T, hw=True, debug=False, dbg_out=False):
    key = (t_steps, hw, debug, dbg_out)
    if key not in _CACHE:
        _CACHE[key] = Enc(t_steps, hw=hw, debug=debug, dbg_out=dbg_out).build()
    return _CACHE[key]


def _assemble(results, t_steps=T):
    xs = np.concatenate([np.asarray(results[s]["oseq"]) for s in range(NCORES)],
                        axis=2).astype(np.float32)
    c = np.concatenate([np.asarray(results[s]["oc"]).T for s in range(NCORES)],
                       axis=1).astype(np.float32)
    h = xs[:, -1, :].copy()
    return xs, h, c


def kernel(tokens, emb, W0, U0, b0, W1, U1, b1, _trace=False, _t_steps=T):
    _install_ntff_hook()
    nc = _get_module(_t_steps, hw=True)
    in_maps = _prep_inputs(tokens, emb, W0, U0, b0, W1, U1, b1, _t_steps)
    res = bass_utils.run_bass_kernel_spmd(
        nc, in_maps, core_ids=list(range(NCORES)), trace=_trace,
        tmpdir='/tmp/enc_trace' if _trace else None)
    x, h, c = _assemble(res.results, _t_steps)
    if _trace:
        kernel.last_exec_ns = res.exec_time_ns
    return (x, h, c)


# revision 15
# speedup vs baseline: 1.1853x; 1.1853x over previous
"""Trainium2 Bass kernel for nn_Encoder (2-layer LSTM encoder).

Strategy: tensor-parallel over the 1024 LSTM units across 8 cores.
Each core owns 128 units (512 of the 4096 gate columns, gate-major
[i|f|g|o]). Per recurrence step each core computes its gate slice
z^T = U_s^T @ h^T  as 8 K-tiles x 4 M-tiles of [128x128]x[128,64]
matmuls, applies the gate nonlinearities in [units, batch] layout,
and broadcasts its new h-shard [128,64] to all peers via remote_dma
(XOR slot addressing, so per-core weights are row-permuted to match).
The input projections (x@W per layer) are done as big parallel GEMMs
before each recurrence, sharded by gate columns (each core computes
its own 512 gate cols for all 16384 (t,b) positions).
"""
import os
import sys
import types
import numpy as np
import ml_dtypes

sys.path.insert(0, '/opt/trn_rl_repo')

from concourse import bass, bass_utils, tile, mybir, bacc  # noqa: E402
from concourse.tile import add_dep_helper  # noqa: E402
import bass_rust  # noqa: E402
from concourse.vector_clock import ScopedClock  # noqa: E402

B, T, E, U, V = 64, 256, 512, 1024, 32000
NCORES = 8
G = 4  # gates
UPC = U // NCORES          # units per core = 128
MPC = G * UPC // 128       # M tiles per core = 4
BT = B * T                 # 16384
AF = mybir.ActivationFunctionType
bf16 = mybir.dt.bfloat16
f32 = mybir.dt.float32


# ---------------------------------------------------------------- tile patch
def _patched_drain_and_barrier(self, tick_clock, wait_clock):
    nc = self.nc
    nops = [nc.sync.nop(nofuse=True, hint=f"drain_wait_{i}") for i in range(24)]
    drain_inst = nc.sync.drain()
    wait_clock.add_sem_waits(
        drain_inst.ins, ScopedClock({None: tick_clock.global_clock})
    )
    si = drain_inst.ins.sync_info
    if si is not None and si.on_wait and len(si.on_wait) > 1:
        waits = list(si.on_wait)
        extra, keep = waits[:-1], waits[-1:]
        assert len(extra) <= len(nops), f"too many drain waits: {len(waits)}"
        for nop, w in zip(nops, extra):
            nop.ins.sync_info = bass_rust.SyncInfo(on_wait=[w], on_update=[])
        drain_inst.ins.sync_info = bass_rust.SyncInfo(
            on_wait=keep, on_update=list(si.on_update)
        )
    nc.all_engine_barrier()
    assert self.sems is not None
    popped = nc._tile_sem_poison_stack.pop()
    assert popped is self._sem_poison
    nc.clear_and_free_semaphores(list(self.sems.allocated().values()))
    nc.all_engine_barrier()


tile.TileContext._drain_and_barrier = _patched_drain_and_barrier


def _install_ntff_hook():
    """Recreate the missing antenv.axon_hooks so trace=True profiling works."""
    try:
        import antenv
        if 'antenv.axon_hooks' in sys.modules:
            return
        mod = types.ModuleType('antenv.axon_hooks')
        holder = [None]
        mod.set_axon_ntff_profile_hook = lambda h: holder.__setitem__(0, h)
        mod.get_axon_ntff_profile_hook = lambda: holder[0]
        sys.modules['antenv.axon_hooks'] = mod
        antenv.axon_hooks = mod
        from trn_agent_boot.trn_boot import _ntff_profile_via_ctypes
        mod.set_axon_ntff_profile_hook(
            _ntff_profile_via_ctypes('/opt/axon/libaxon_pjrt.so'))
    except Exception:
        pass


# ---------------------------------------------------------------- builder
class Enc:
    """Builds the SPMD device program."""

    def __init__(self, t_steps=T, hw=True, debug=False, dbg_out=False):
        self.T = t_steps
        self.hw = hw  # apply cross-die xor2 routing compensation
        self.dbg_out = dbg_out
        self.deferred = []  # (inst, sem, value) waits injected post-schedule
        nc = self.nc = bacc.Bacc("TRN2", target_bir_lowering=False,
                                 debug=debug, num_devices=NCORES)
        BT_ = B * self.T
        dt = nc.dram_tensor
        # inputs (per-core values differ; same names/shapes)
        self.emb = dt("emb", [V, E], bf16, kind="ExternalInput")
        self.toki = dt("toki", [128, BT_ // 16], mybir.dt.int16, kind="ExternalInput")
        self.w0 = dt("w0", [E // 128, 128, G * UPC], bf16, kind="ExternalInput")
        self.w1 = dt("w1", [U // 128, 128, G * UPC], bf16, kind="ExternalInput")
        self.u0 = dt("u0", [U // 128, 128, G * UPC], bf16, kind="ExternalInput")
        self.u1 = dt("u1", [U // 128, 128, G * UPC], bf16, kind="ExternalInput")
        self.b0 = dt("b0", [128, MPC], f32, kind="ExternalInput")
        self.b1 = dt("b1", [128, MPC], f32, kind="ExternalInput")
        self.ident = dt("ident", [128, 128], bf16, kind="ExternalInput")
        # internal DRAM (ExternalOutput in debug builds for inspection)
        ik = dict(kind="ExternalOutput") if dbg_out else {}
        self.xz0 = dt("xz0", [MPC, 128, BT_], f32, **ik)
        self.x1t = dt("x1t", [NCORES, 128, BT_], bf16, **ik)
        self.xz1 = dt("xz1", [MPC, 128, BT_], f32, **ik)
        # outputs
        self.oseq = dt("oseq", [B, self.T, UPC], f32, kind="ExternalOutput")
        self.oc = dt("oc", [128, B], f32, kind="ExternalOutput")

        # static SBUF
        sb = nc.alloc_sbuf_tensor
        self.u_sb = [sb("u0_sb", [128, NCORES, G * UPC], bf16),
                     sb("u1_sb", [128, NCORES, G * UPC], bf16)]
        self.w0_sb = sb("w0_sb", [128, E // 128, G * UPC], bf16)
        self.w1_sb = sb("w1_sb", [128, U // 128, G * UPC], bf16)
        self.b_sb = [sb("b0_sb", [128, MPC], f32), sb("b1_sb", [128, MPC], f32)]
        self.id_sb = sb("id_sb", [128, 128], bf16)
        self.toki_sb = sb("toki_sb", [128, (B * self.T) // 16], mybir.dt.int16)
        self.hs = sb("hs", [128, 4, NCORES, B], bf16)   # [u, ring, slot, b]
        self.cs = sb("cs", [128, 2, B], f32)
        # cross-core sems
        self.rsem = [[nc.alloc_semaphore(f"rs{l}_{d}") for d in range(8)]
                     for l in range(2)]
        self.lsem = [nc.alloc_semaphore("ls0"), nc.alloc_semaphore("ls1")]

    def wait_carrier(self, engine, sem, val, anchor, hint):
        n = engine.nop(nofuse=True, hint=hint)
        if anchor is not None:
            add_dep_helper(n.ins, anchor.ins, sync=False, reason="carrier-anchor")
        self.deferred.append((n, sem, val))
        return n

    # ------------------------------------------------------------ phases
    def load_consts(self):
        nc = self.nc
        nc.sync.dma_start(out=self.u_sb[0].ap(),
                          in_=self.u0.ap().rearrange("k p m -> p k m"))
        nc.sync.dma_start(out=self.u_sb[1].ap(),
                          in_=self.u1.ap().rearrange("k p m -> p k m"))
        nc.sync.dma_start(out=self.w0_sb.ap(),
                          in_=self.w0.ap().rearrange("k p m -> p k m"))
        nc.sync.dma_start(out=self.w1_sb.ap(),
                          in_=self.w1.ap().rearrange("k p m -> p k m"))
        nc.sync.dma_start(out=self.b_sb[0].ap(), in_=self.b0.ap())
        nc.sync.dma_start(out=self.b_sb[1].ap(), in_=self.b1.ap())
        nc.sync.dma_start(out=self.id_sb.ap(), in_=self.ident.ap())
        nc.sync.dma_start(out=self.toki_sb.ap(), in_=self.toki.ap())
        nc.vector.memset(self.hs.ap()[:, 0, :, :], 0.0)
        nc.vector.memset(self.cs.ap()[:, 0, :], 0.0)

    def phase_in_proj(self, ctx, tc, layer):
        """xz[l]^T[m][128, BT] = W^T x^T + b for this core's gate cols."""
        nc = self.nc
        BT_ = B * self.T
        CH = min(512, BT_)
        nch = BT_ // CH
        kt = E // 128 if layer == 0 else U // 128
        w_sb = self.w0_sb if layer == 0 else self.w1_sb
        xz = self.xz0 if layer == 0 else self.xz1
        rhs_pool = ctx.enter_context(tc.tile_pool(name=f"ph{layer}_rhs", bufs=3))
        out_pool = ctx.enter_context(tc.tile_pool(name=f"ph{layer}_out", bufs=2))
        ps_pool = ctx.enter_context(
            tc.tile_pool(name=f"ph{layer}_ps", bufs=2, space="PSUM"))
        for c in range(nch):
            if layer == 0:
                rhs = rhs_pool.tile([128, kt, CH], bf16, tag="rhs")
                nc.gpsimd.dma_gather(
                    out_ap=rhs[:, :, :],
                    in_ap=self.emb.ap(),
                    idxs_ap=self.toki_sb.ap()[:, (CH // 16) * c:(CH // 16) * (c + 1)],
                    num_idxs=CH,
                    num_idxs_reg=CH,
                    elem_size=E,
                    transpose=True,
                )
            else:
                rhs = rhs_pool.tile([128, kt, CH], bf16, tag="rhs")
                nc.sync.dma_start(
                    out=rhs[:, :, :],
                    in_=self.x1t.ap().rearrange("k p n -> p k n")[:, :, CH * c:CH * (c + 1)],
                )
            out_t = out_pool.tile([128, MPC, CH], f32, tag="out")
            for m in range(MPC):
                ps = ps_pool.tile([128, CH], f32, tag=f"ps{m}", name=f"ps{m}")
                for k in range(kt):
                    nc.tensor.matmul(
                        ps[:, :], w_sb.ap()[:, k, 128 * m:128 * (m + 1)],
                        rhs[:, k, :], start=(k == 0), stop=(k == kt - 1))
                nc.scalar.activation(out_t[:, m, :], ps[:, :], AF.Identity,
                                     bias=self.b_sb[layer].ap()[:, m:m + 1])
            nc.sync.dma_start(
                out=xz.ap()[:, :, CH * c:CH * (c + 1)].rearrange("m p n -> p m n"),
                in_=out_t[:, :, :])

    def recurrence(self, ctx, tc, layer):
        nc = self.nc
        Tn = self.T
        xz = self.xz0 if layer == 0 else self.xz1
        u_sb = self.u_sb[layer]
        rsem, lsem = self.rsem[layer], self.lsem[layer]
        hs, cs = self.hs.ap(), self.cs.ap()

        xz_pool = ctx.enter_context(tc.tile_pool(name=f"r{layer}_xz", bufs=4))
        z_pool = ctx.enter_context(tc.tile_pool(name=f"r{layer}_z", bufs=2))
        a_pool = ctx.enter_context(tc.tile_pool(name=f"r{layer}_a", bufs=2))
        t_pool = ctx.enter_context(tc.tile_pool(name=f"r{layer}_t", bufs=2))
        o_pool = ctx.enter_context(tc.tile_pool(name=f"r{layer}_o", bufs=3))
        ps_pool = ctx.enter_context(
            tc.tile_pool(name=f"r{layer}_ps", bufs=2, space="PSUM"))
        tr_pool = ctx.enter_context(
            tc.tile_pool(name=f"r{layer}_tr", bufs=2, space="PSUM"))

        last_pe = None       # anchor chain on PE
        last_dve_w = None    # anchor for DVE lsem carrier
        last_sp = None       # last sync-engine instruction (anchor)
        st_q = []            # pending x1t stores (gate trigger 2 steps later)
        PR = 4               # h slot ring depth
        for t in range(Tn):
            par, nxt = t % PR, (t + 1) % PR
            cpar, cnxt = t % 2, (t + 1) % 2
            xz_t = xz_pool.tile([128, MPC, B], f32, tag="xz")
            nc.sync.dma_start(out=xz_t[:, :, :],
                              in_=xz.ap()[:, :, B * t:B * (t + 1)].rearrange("m p n -> p m n"))
            ps = ps_pool.tile([128, MPC * B], f32, tag="ps")
            pend_carrier = False
            for m in range(MPC):
                for d in range(NCORES):
                    if m == 0 and d > 0 and t > 0:
                        w = self.wait_carrier(nc.tensor, rsem[d], 2 * t, last_pe,
                                              f"r{layer}t{t}d{d}")
                        last_pe = w
                        pend_carrier = True
                    mm = nc.tensor.matmul(
                        ps[:, B * m:B * (m + 1)],
                        u_sb.ap()[:, d, 128 * m:128 * (m + 1)],
                        hs[:, par, d, :],
                        start=(d == 0), stop=(d == NCORES - 1))
                    if pend_carrier:
                        add_dep_helper(mm.ins, last_pe.ins, sync=False,
                                       reason="mm-after-carrier")
                        pend_carrier = False
                    last_pe = mm
            # z = psum + xz ; i,f,g,o each [128, B]
            z = z_pool.tile([128, MPC * B], f32, tag="z")
            nc.vector.tensor_add(z[:, :], ps[:, :],
                                 xz_t[:, :, :].rearrange("p m n -> p (m n)"))
            a = a_pool.tile([128, MPC * B], f32, tag="a")
            nc.scalar.activation(a[:, 0:2 * B], z[:, 0:2 * B], AF.Sigmoid)
            nc.scalar.activation(a[:, 2 * B:3 * B], z[:, 2 * B:3 * B], AF.Tanh)
            nc.scalar.activation(a[:, 3 * B:4 * B], z[:, 3 * B:4 * B], AF.Sigmoid)
            t1 = t_pool.tile([128, B], f32, tag="t1")
            t2 = t_pool.tile([128, B], f32, tag="t2")
            tc_t = t_pool.tile([128, B], f32, tag="tc")
            nc.vector.tensor_mul(t1[:, :], a[:, B:2 * B], cs[:, cpar, :])
            nc.vector.tensor_mul(t2[:, :], a[:, 0:B], a[:, 2 * B:3 * B])
            nc.vector.tensor_add(cs[:, cnxt, :], t1[:, :], t2[:, :])
            nc.scalar.activation(tc_t[:, :], cs[:, cnxt, :], AF.Tanh)
            if t >= PR:
                # protect send source buffer from overwrite while in flight
                w = self.wait_carrier(nc.vector, lsem, 112 * (t - PR + 1),
                                      last_dve_w, f"l{layer}ls{t}")
                hw_inst = nc.vector.tensor_mul(hs[:, nxt, 0, :], a[:, 3 * B:4 * B],
                                               tc_t[:, :])
                add_dep_helper(hw_inst.ins, w.ins, sync=False, reason="h-after-lsem")
            else:
                hw_inst = nc.vector.tensor_mul(hs[:, nxt, 0, :], a[:, 3 * B:4 * B],
                                               tc_t[:, :])
            last_dve_w = hw_inst
            # layer0: store gathered h^{t} (parity par, all slots valid after
            # this step's MMs) to x1t. The trigger below is gated on this
            # store so no peer's h^{t+2} can land before it drains.
            st = None
            if layer == 0 and t >= 1:
                st = nc.sync.dma_start(
                    out=self.x1t.ap()[:, :, B * (t - 1):B * t].rearrange("k p n -> p k n"),
                    in_=hs[:, par, :, :])
                add_dep_helper(st.ins, last_pe.ins, sync=True,
                               reason="x1t-after-mms")
                last_sp = st
            # broadcast own shard to peers' slot d
            for d in range(1, NCORES):
                rd = [None] * 8
                tgt = (d ^ 2) if (self.hw and d >= 4) else d
                rd[d] = (0, tgt)
                nc.gpsimd.remote_dma_broadcast(
                    out_ap=hs[:, nxt, d, :], in_ap=hs[:, nxt, 0, :],
                    remote_sem=rsem[d], local_sem=lsem, rdests=rd)
            if st is not None:
                st_q.append(st)
            trig = nc.gpsimd.trigger_dma(count=None)
            if len(st_q) >= 3:
                add_dep_helper(trig.ins, st_q[-3].ins, sync=True,
                               reason="sends-after-x1t-store-lagged")
            if layer != 0:
                # transpose own new h-shard -> [B, UPC] and store to oseq
                tr = tr_pool.tile([B, 128], bf16, tag="tr")
                nc.tensor.transpose(tr[:, :], hs[:, nxt, 0, :], self.id_sb.ap())
                ot = o_pool.tile([B, 128], f32, tag="ot")
                nc.vector.tensor_copy(ot[:, :], tr[:, :])
                nc.sync.dma_start(out=self.oseq.ap()[:, t, :], in_=ot[:, :])

        # final gathered h^{T} for x1t (layer 0) needs the last exchange
        if layer == 0:
            par = Tn % PR
            anchor = last_sp
            for d in range(1, NCORES):
                anchor = self.wait_carrier(nc.sync, self.rsem[0][d], 2 * Tn,
                                           anchor, f"fin0_{d}")
            st = nc.sync.dma_start(
                out=self.x1t.ap()[:, :, B * (Tn - 1):B * Tn].rearrange("k p n -> p k n"),
                in_=hs[:, par, :, :])
            add_dep_helper(st.ins, anchor.ins, sync=False, reason="fin-x1t")
            # reset state for layer 1 (after the store)
            ms1 = nc.vector.memset(hs[:, 0, :, :], 0.0)
            add_dep_helper(ms1.ins, st.ins, sync=True, reason="reset-after-store")
            nc.vector.memset(cs[:, 0, :], 0.0)
        else:
            nc.sync.dma_start(out=self.oc.ap(), in_=cs[:, Tn % 2, :])

    def build(self):
        from contextlib import ExitStack
        nc = self.nc
        with tile.TileContext(nc) as tc:
            self.load_consts()
            with ExitStack() as ctx:
                self.phase_in_proj(ctx, tc, 0)
            with ExitStack() as ctx:
                self.recurrence(ctx, tc, 0)
            with ExitStack() as ctx:
                self.phase_in_proj(ctx, tc, 1)
            with ExitStack() as ctx:
                self.recurrence(ctx, tc, 1)
        for inst, sem, val in self.deferred:
            if val > 0:
                bass_rust.wait_op(inst.ins, sem, val, "sem-ge", True)
        nc.compile()
        return nc


# ---------------------------------------------------------------- host side
def _prep_inputs(tokens, emb, W0, U0, b0, W1, U1, b1, t_steps=T):
    tokens = np.asarray(tokens)
    emb = np.asarray(emb, np.float32)
    W0 = np.asarray(W0, np.float32); U0 = np.asarray(U0, np.float32)
    W1 = np.asarray(W1, np.float32); U1 = np.asarray(U1, np.float32)
    b0 = np.asarray(b0, np.float32); b1 = np.asarray(b1, np.float32)
    emb_bf = emb.astype(ml_dtypes.bfloat16)
    seq = tokens[:, :t_steps].T.reshape(-1).astype(np.int16)  # col j=(t*B+b)
    toki = np.zeros((128, seq.size // 16), np.int16)
    toki[:16] = seq.reshape(-1, 16).T
    toki[16:32] = toki[:16]  # HW SWDGE ucode reads rows 16-31; sim reads 0-15
    ident = np.eye(128, dtype=ml_dtypes.bfloat16)

    def gate_cols(s):
        # column indices of core s's 512 gate cols, gate-major
        return np.concatenate(
            [np.arange(g * U + UPC * s, g * U + UPC * (s + 1)) for g in range(G)])

    in_maps = []
    for s in range(NCORES):
        cols = gate_cols(s)
        w0s = W0[:, cols]                      # [512, 512]
        w0t = np.ascontiguousarray(
            w0s.reshape(E // 128, 128, G * UPC)).astype(ml_dtypes.bfloat16)
        # xor-permuted K tiles for U and W1
        def xperm(Wm):
            out = np.empty((U // 128, 128, G * UPC), np.float32)
            for j in range(NCORES):
                src = Wm[UPC * (s ^ j):UPC * ((s ^ j) + 1), cols]
                out[j] = src
            return out.astype(ml_dtypes.bfloat16)
        u0s = xperm(U0)
        u1s = xperm(U1)
        w1s = xperm(W1)
        b0s = np.ascontiguousarray(b0[cols].reshape(G, UPC).T.reshape(128, MPC),)
        # careful: cols grouped gate-major: b[cols] = [i(128)|f|g|o];
        # b_sb[u, m] = bias of gate m unit u  -> reshape(G, UPC).T is [UPC, G]
        b1s = np.ascontiguousarray(b1[cols].reshape(G, UPC).T.reshape(128, MPC))
        in_maps.append({
            "emb": emb_bf, "toki": toki, "ident": ident,
            "w0": w0t, "w1": w1s, "u0": u0s, "u1": u1s,
            "b0": b0s.astype(np.float32), "b1": b1s.astype(np.float32),
        })
    return in_maps


_CACHE = {}


def _get_module(t_steps=T, hw=True, debug=False, dbg_out=False):
    key = (t_steps, hw, debug, dbg_out)
    if key not in _CACHE:
        _CACHE[key] = Enc(t_steps, hw=hw, debug=debug, dbg_out=dbg_out).build()
    return _CACHE[key]


def _assemble(results, t_steps=T):
    xs = np.concatenate([np.asarray(results[s]["oseq"]) for s in range(NCORES)],
                        axis=2).astype(np.float32)
    c = np.concatenate([np.asarray(results[s]["oc"]).T for s in range(NCORES)],
                       axis=1).astype(np.float32)
    h = xs[:, -1, :].copy()
    return xs, h, c


def kernel(tokens, emb, W0, U0, b0, W1, U1, b1, _trace=False, _t_steps=T):
    _install_ntff_hook()
    if _trace:
        import shutil, tempfile
        shutil.rmtree('/tmp/enc_trace', ignore_errors=True)
    nc = _get_module(_t_steps, hw=True)
    in_maps = _prep_inputs(tokens, emb, W0, U0, b0, W1, U1, b1, _t_steps)
    res = bass_utils.run_bass_kernel_spmd(
        nc, in_maps, core_ids=list(range(NCORES)), trace=_trace,
        tmpdir='/tmp/enc_trace' if _trace else None)
    x, h, c = _assemble(res.results, _t_steps)
    if _trace:
        kernel.last_exec_ns = res.exec_time_ns
    return (x, h, c)
